# revision 8
# baseline (speedup 1.0000x reference)
"""CPPN forward (12-layer tiny MLP over 4.2M pixels) on 8 TRN2 NeuronCores.

Strategy (pure data parallel, per sharding hint):
- Pixels sharded 8 ways; per core 524288 px padded to 208 supertiles (ST).
- One ST = 5 pixel-blocks x 512 px. Feature channels live on SBUF/PSUM
  partitions: 5 blocks block-diagonally packed into one 128-wide matmul
  (110 rows: 85 identity-ish channels, 20 gaussian, 5 sin).
- Per layer per ST: one matmul [K<=110, M=110, N=512] (lhsT = permuted
  block-diag weights, built host-side), then one wide DVE transit
  (PSUM->SBUF + per-partition bias = the folded "-1" of 2*exp(-h^2)-1),
  then narrow ACT ops: Square+Exp for gaussian rows, Sin2pi for sin rows.
- sin(h): ACT's Sin2pi spline is only accurate to |h|~3.4. Layers whose
  sin pre-activation exceeds that use the triple-angle identity
  sin(h) = -4*s*(s^2-0.75), s = sin(h/3) = Sin2pi(h/(6pi)); the -4 is
  folded into the next layer's weights.
- The gaussian act 2*exp(-h^2)-1 is folded as: rows x2 in the next
  weights, -sum(gauss rows) as a per-partition bias added in the transit.
- Sin2pi lives in the exp_and_friends ACT table set together with
  Exp/Square/Identity/Copy -> a single table load, no switching. mybir
  has no Sin2pi enum, so activations are emitted as Sin and the
  serialized BIR JSON is patched Sin->Sin2pi before compilation.
"""
import sys, types
import numpy as np

sys.path.insert(0, "/opt/trn_rl_repo")

# ---------------------------------------------------------------- constants
N_PIX = 2048 * 2048
D_IN, D_HID, D_OUT = 4, 22, 3
N_HIDDEN = 11
N_CORES = 8
FD = 512                      # pixels per block (= matmul free dim)
BLOCKS = 5                    # blocks per supertile (5*22=110 partitions)
ST_PX = BLOCKS * FD           # 2560 px per supertile
GROUP = 4                     # supertiles per group (PSUM banks / transit width)
PX_CORE = N_PIX // N_CORES            # 524288
N_ST = -(-PX_CORE // ST_PX)           # 205
N_GROUP = -(-N_ST // GROUP)           # 52
N_ST_PAD = N_GROUP * GROUP            # 208
PX_PAD = N_ST_PAD * ST_PX             # 532480

ID_CH = list(range(15)) + [19, 20]    # 17 identity channels per block
GA_CH = [15, 16, 17, 18]
SI_CH = [21]
N_ID, N_GA, N_SI = 85, 20, 5          # *5 blocks
# engine partition bases must be 32-aligned -> layout:
#   rows 0..63   id channels 0..63
#   rows 64..68  sin (base 64)
#   rows 69..89  id channels 64..84
#   rows 90..95  zero pad
#   rows 96..115 gauss (base 96)
ROWS = 116
SIN0, GA0 = 64, 96
TWO_PI = 2.0 * np.pi

# The Sin2pi spline is only accurate to ~0.55 turns (|h| ~ 3.4). Observed
# sin pre-activations reach |h| ~ 7.3 (and vary with the harness PRNG
# backend), so ALL layers use the triple-angle form, valid to |h| ~ 10.5.
TRIPLE = set(range(1, 13))

# ------------------------------------------------------------- host packing
def _row_of(b, c):
    """partition row of (block b, original channel c) in the ST layout"""
    if c in GA_CH:
        return GA0 + b * 4 + (c - 15)
    if c == 21:
        return SIN0 + b
    g = b * 17 + ID_CH.index(c)
    return g if g < 64 else 69 + (g - 64)

_ROW_BC = [(b, c) for b in range(BLOCKS) for c in range(D_HID)]

def _in_scale(c, act_layer):
    """fold factor applied to weight rows that consume act outputs"""
    if c in GA_CH:
        return 2.0
    if c == 21 and act_layer in TRIPLE:
        return -4.0
    return 1.0

def pack_weights(W_in, W_hidden, W_out):
    """Build the 13 block-diagonal lhsT matrices + transit bias vectors."""
    W_in, W_hidden, W_out = (np.asarray(W_in, np.float32),
                             np.asarray(W_hidden, np.float32),
                             np.asarray(W_out, np.float32))
    # MM1: x -> layer1 preact. lhsT [20, 110]
    lin = np.zeros((BLOCKS * 4, ROWS), np.float32)
    for b in range(BLOCKS):
        for ci in range(D_IN):
            for co in range(D_HID):
                lin[b * 4 + ci, _row_of(b, co)] = W_in[ci, co]
    # MM2..12: hidden. lhsT [110, 110]; bias [110]
    lh = np.zeros((N_HIDDEN, ROWS, ROWS), np.float32)
    # cols 0..12: transit bias of MM j+1 (col 0 zero).
    # cols 13..25: same bias / 2pi (sin rows only) for direct-sin act bias.
    bias = np.zeros((ROWS, 26), np.float32)
    for i in range(N_HIDDEN):
        a = i + 1                              # act layer consumed by this MM
        W = W_hidden[i]
        for b in range(BLOCKS):
            for ci in range(D_HID):
                s = _in_scale(ci, a)
                ri = _row_of(b, ci)
                for co in range(D_HID):
                    lh[i, ri, _row_of(b, co)] = W[ci, co] * s
        bvec = -W[15:19, :].sum(axis=0)        # per output channel
        for b in range(BLOCKS):
            for co in range(D_HID):
                bias[_row_of(b, co), i + 1] = bvec[co]
                bias[_row_of(b, co), 13 + i + 1] = bvec[co] / TWO_PI
    # MM13: out. lhsT [110, 15] (+ obias on the packed [111] out layout)
    lo = np.zeros((ROWS, BLOCKS * 3), np.float32)
    for b in range(BLOCKS):
        for ci in range(D_HID):
            s = _in_scale(ci, 12)
            for co in range(D_OUT):
                lo[_row_of(b, ci), b * 3 + co] = W_out[ci, co] * s
    bo = -W_out[15:19, :].sum(axis=0)          # [3]
    obias = np.zeros((111, 1), np.float32)
    for r in range(GROUP):
        for b in range(BLOCKS):
            for co in range(D_OUT):
                obias[32 * r + b * 3 + co, 0] = bo[co]
    return {"w_in": lin, "w_hid": lh, "w_out": lo, "bias": bias, "obias": obias}

def pack_x(x):
    """[N_PIX,4] -> per-core [52, 20, 4, 512] f32 arrays."""
    x = np.asarray(x, np.float32)
    out = []
    for k in range(N_CORES):
        shard = x[k * PX_CORE:(k + 1) * PX_CORE]
        pad = np.zeros((PX_PAD, D_IN), np.float32)
        pad[:PX_CORE] = shard
        a = pad.reshape(N_GROUP, GROUP, BLOCKS, FD, D_IN)
        a = a.transpose(0, 2, 4, 1, 3).reshape(N_GROUP, BLOCKS * D_IN, GROUP, FD)
        out.append(np.ascontiguousarray(a))
    return out

_OUT_ROWS = np.array([[32 * r + b * 3 + co for b in range(BLOCKS) for co in range(D_OUT)]
                      for r in range(GROUP)])  # [4, 15]

def unpack_out(outs):
    """per-core [52, 111, 512] -> [N_PIX, 3] f32"""
    full = np.empty((N_PIX, D_OUT), np.float32)
    for k, od in enumerate(outs):
        g = od[:, _OUT_ROWS.reshape(-1), :]                     # [52, 60, 512]
        g = g.reshape(N_GROUP, GROUP, BLOCKS, D_OUT, FD)
        g = g.transpose(0, 1, 2, 4, 3).reshape(PX_PAD, D_OUT)   # [532480, 3]
        full[k * PX_CORE:(k + 1) * PX_CORE] = g[:PX_CORE]
    return full

# ------------------------------------------------------------ device kernel
_CACHE = {}

def _shim_hooks():
    import antenv
    if "antenv.axon_hooks" in sys.modules:
        return
    hooks = types.ModuleType("antenv.axon_hooks")
    hooks._hook = None
    hooks.set_axon_ntff_profile_hook = lambda h: setattr(hooks, "_hook", h)
    hooks.get_axon_ntff_profile_hook = lambda: hooks._hook
    sys.modules["antenv.axon_hooks"] = hooks
    antenv.axon_hooks = hooks
    try:
        from trn_agent_boot.trn_boot import _ntff_profile_via_ctypes
        hooks._hook = _ntff_profile_via_ctypes("/opt/axon/libaxon_pjrt.so")
    except Exception:
        pass

def _build():
    _shim_hooks()
    import concourse.bacc as bacc_mod
    import concourse.mybir as mybir
    import concourse.tile as tile
    from concourse.hw_specs import get_activation_tables as _real_gat

    AFT = mybir.ActivationFunctionType
    ours = {AFT.Square, AFT.Exp, AFT.Identity, AFT.Copy, AFT.Sin, AFT.Relu}

    def _doctored_gat(arch):
        tabs = dict(_real_gat(arch))
        return {n: (set(f) | ours if n == "exp_and_friends" else set(f) - ours)
                for n, f in tabs.items()}

    bacc_mod.get_activation_tables = _doctored_gat

    dt = mybir.dt.float32
    nc = bacc_mod.Bacc(None, target_bir_lowering=False, debug=False)
    x_d = nc.declare_dram_parameter("x", [N_GROUP, 20, GROUP, FD], dt, isOutput=False)
    win_d = nc.declare_dram_parameter("w_in", [20, ROWS], dt, isOutput=False)
    wh_d = nc.declare_dram_parameter("w_hid", [N_HIDDEN, ROWS, ROWS], dt, isOutput=False)
    wo_d = nc.declare_dram_parameter("w_out", [ROWS, 15], dt, isOutput=False)
    b_d = nc.declare_dram_parameter("bias", [ROWS, 26], dt, isOutput=False)
    ob_d = nc.declare_dram_parameter("obias", [111, 1], dt, isOutput=False)
    o_d = nc.declare_dram_parameter("out", [N_GROUP, 111, FD], dt, isOutput=True)

    with tile.TileContext(nc) as tc:
        with (tc.tile_pool(name="wpool", bufs=1) as wpool,
              tc.tile_pool(name="xpool", bufs=3) as xpool,
              tc.tile_pool(name="hpool", bufs=3) as hpool,
              tc.tile_pool(name="tpool", bufs=2) as tpool,
              tc.tile_pool(name="opool", bufs=3) as opool,
              tc.tile_pool(name="ppool", bufs=2, space="PSUM") as ppool):
            win = wpool.tile([20, ROWS], dt)
            wh = [wpool.tile([ROWS, ROWS], dt, tag=f"wh{i}", name=f"wh{i}") for i in range(N_HIDDEN)]
            wo = wpool.tile([ROWS, 15], dt)
            bt = wpool.tile([ROWS, 26], dt)
            ob = wpool.tile([111, 1], dt)
            nc.sync.dma_start(out=win[:], in_=win_d[:])
            for i in range(N_HIDDEN):
                nc.sync.dma_start(out=wh[i][:], in_=wh_d[i])
            nc.sync.dma_start(out=wo[:], in_=wo_d[:])
            nc.sync.dma_start(out=bt[:], in_=b_d[:])
            nc.sync.dma_start(out=ob[:], in_=ob_d[:])

            for g in range(N_GROUP):
                xg = xpool.tile([20, GROUP, FD], dt, tag="xg")
                nc.sync.dma_start(out=xg[:], in_=x_d[g])

                H = None
                for mm in range(1, 14):           # 13 matmul rounds
                    if mm == 13:
                        O = ppool.tile([111, FD], dt, tag="pm")
                        for r in range(GROUP):
                            nc.tensor.matmul(O[32 * r:32 * r + 15, :], wo[:],
                                             H[:, r, :], start=True, stop=True,
                                             tile_position=(0, 32 * r))
                        ot = opool.tile([111, FD], dt, tag="ot")
                        nc.vector.tensor_scalar_add(ot[:], O[:], ob[:])
                        nc.sync.dma_start(out=o_d[g], in_=ot[:])
                        break
                    P = ppool.tile([ROWS, GROUP, FD], dt, tag="pm")
                    for r in range(GROUP):
                        if mm == 1:
                            nc.tensor.matmul(P[:, r, :], win[:], xg[:, r, :],
                                             start=True, stop=True)
                        else:
                            nc.tensor.matmul(P[:, r, :], wh[mm - 2][:],
                                             H[:, r, :], start=True, stop=True)
                    # wide transit: H = P + bias (covers id/gauss/sin rows)
                    H = hpool.tile([ROWS, GROUP, FD], dt, tag="H")
                    nc.vector.tensor_scalar_add(H[:, :, :], P[:, :, :],
                                                bt[:, mm - 1:mm])
                    a = mm                         # act layer index 1..12
                    # gaussian rows: t = exp(-h^2)
                    sq = tpool.tile([N_GA, GROUP, FD], dt, tag="sq")
                    nc.vector.scalar_tensor_tensor(
                        sq[:], H[GA0:GA0 + 20, :, :], 0.0, H[GA0:GA0 + 20, :, :],
                        op0=mybir.AluOpType.add, op1=mybir.AluOpType.mult)
                    nc.scalar.activation(H[GA0:GA0 + 20, :, :], sq[:], AFT.Exp,
                                         bias=0.0, scale=-1.0)
                    # sin rows
                    if a in TRIPLE:
                        s1 = tpool.tile([N_SI, GROUP, FD], dt, tag="s1")
                        s2 = tpool.tile([N_SI, GROUP, FD], dt, tag="s2")
                        nc.scalar.activation(s1[:], H[SIN0:SIN0 + 5, :, :], AFT.Sin,
                                             bias=0.0, scale=1.0 / (3 * TWO_PI))
                        nc.scalar.activation(s2[:], s1[:], AFT.Square,
                                             bias=0.0, scale=1.0)
                        nc.vector.scalar_tensor_tensor(
                            H[SIN0:SIN0 + 5, :, :], s2[:], 0.75, s1[:],
                            op0=mybir.AluOpType.subtract, op1=mybir.AluOpType.mult)
                    else:
                        nc.scalar.activation(H[SIN0:SIN0 + 5, :, :], P[SIN0:SIN0 + 5, :, :],
                                             AFT.Sin, bias=bt[SIN0:SIN0 + 5, 13 + mm - 1:13 + mm],
                                             scale=1.0 / TWO_PI)
    nc.compile()

    _orig = nc.to_json_bytes
    nc.to_json_bytes = lambda: _orig().replace(b'"func":"Sin"', b'"func":"Sin2pi"')
    return nc

def _get_nc():
    if "nc" not in _CACHE:
        _CACHE["nc"] = _build()
    return _CACHE["nc"]

def run_device(x_cores, w):
    from concourse.bass_utils import run_bass_kernel_spmd
    nc = _get_nc()
    in_maps = [{"x": x_cores[k], "w_in": w["w_in"], "w_hid": w["w_hid"],
                "w_out": w["w_out"], "bias": w["bias"], "obias": w["obias"]}
               for k in range(N_CORES)]
    res = run_bass_kernel_spmd(nc, in_maps, list(range(N_CORES)), trace=False)
    return [res.results[k]["out"] for k in range(N_CORES)]

def kernel(x, W_in, W_hidden, W_out):
    w = pack_weights(W_in, W_hidden, W_out)
    x_cores = pack_x(x)
    outs = run_device(x_cores, w)
    return unpack_out(outs)
